# revision 46
# baseline (speedup 1.0000x reference)
"""Trainium2 Bass kernel for nn_NeuralALU (batched byte-encoded 32-bit add).

The reference network computes, per batch element, a chain of table-lookup
matmuls + sharp softmaxes (scale=100) over exactly-one-hot byte encodings.
Because the inputs are exact one-hots, the float pipeline collapses to a
discrete algorithm (validated to ~1e-22 rel-err):

  xl = (a%16 + b%16), xh = (a>>4 + b>>4)           per byte, in [0,30]
  carry state c in {0, 0.5, 1}, init 0.5, over 8 nibbles (lo0,hi0,...,hi3):
      add = (c == 1); y = x + add; U = y mod 16; P = (c == 0.5)
      c' = clamp(x + c - 15, 0, 1)
  nibble dist = onehot(U)*(1-P/2) + onehot((U+1) mod 16)*(P/2)
  out byte row [256] = outer(h_dist, l_dist) flattened

v4 architecture:
  - Input staged host-side as fp8-e4m3 (one-hots {0,1} are exact) and
    TRANSPOSED to [2048, 4096] per core.  8 MiB/core input traffic.
  - Nibble-sum extraction on the TensorEngine: stationary [128, 8] code
    tables (lo/hi nibble value per one-hot position), moving [128, 512]
    input columns, f32 PSUM accumulation over 16 k-chunks.  The a- and
    b-halves sum in the same accumulation, so PSUM IS xlo/xhi per byte.
  - [8, 512] PSUM results cast to bf16 and PE-transposed to row-major
    [128, 8] via tiny identity matmuls.
  - Variable chunking [4,8,8,8,4] row-tiles: small first chunk cuts the
    pipeline-fill latency before DVE starts; small last chunk cuts the
    serial tail after the input stream ends.
  - DVE: carry chain, U/P/weights, chunk-wide dist build in bf16 with
    layout [par, k, tile*byte] -- packed last dims give the 2x DVE mode,
    AND the downstream outer-product reads stay at <=64B inner stride
    (128B strides measured ~1.7x slower).  One merged [128, 4096] outer
    TT per 4-tile quad.
  - Output written bf16 (values in {0,.25,.5,1}, exact), upcast host-side.
  - Input DMAs on the SP HWDGE queue; PSUM evacs + output DMAs on the Act
    queue, output issues deferred so they never head-of-line block an evac.

Sharding: pure data parallel over the batch dim across 8 NeuronCores.
"""

import numpy as np
import ml_dtypes

import concourse.bass as bass
import concourse.bacc as bacc
import concourse.mybir as mybir
from concourse.tile import TileContext
from concourse.bass_utils import run_bass_kernel_spmd

N_CORES = 8
B_FULL = 32768
ROWS = B_FULL // N_CORES   # 4096 rows per core
P = 128
FIN = 2048                 # a|b one-hot columns, concatenated
KC = FIN // P              # 16 k-chunks
GR = 512                   # rows per matmul group (max moving free dim)
NG = ROWS // GR            # 8 groups
TPG = GR // P              # 4 row-tiles per group
# chunks as lists of group row-counts: small chunks at the ends cut the
# pipeline-fill latency (first zt ready sooner) and the serial tail
# (last quad DMA is smaller); 512-row groups in the middle for efficiency
CHUNKS = [[256], [256], [512], [512, 512], [512, 512], [512], [256], [256]]
FOUT = 1024                # 4 bytes x 256 output row

FP = mybir.dt.float32
BF = mybir.dt.bfloat16
F8 = mybir.dt.float8e4
BF_NP = ml_dtypes.bfloat16
F8_NP = ml_dtypes.float8_e4m3

MAX_NTC = max(sum(g) for g in CHUNKS) // P
assert sum(sum(g) for g in CHUNKS) == ROWS


def _const_tables():
    # Code table: for k-chunk c, wtab[p, 8c + 2i + s] = nibble value (lo if
    # s=0 else hi) of one-hot position f = 128c+p, where i is the byte
    # index of f within its a/b half.  Values 0..15: fp8-e4m3-exact.
    W = np.zeros((P, 8 * KC), np.float32)
    f = np.arange(FIN)
    fb = f % 1024
    i_b = fb // 256
    k = fb % 256
    c = f // P
    p = f % P
    W[p, 8 * c + 2 * i_b] = k & 15
    W[p, 8 * c + 2 * i_b + 1] = k >> 4
    # Padded compare table, layout (k, n, t): iota_rep[p, (k,n,t)] =
    # (k-1) mod 16 for k in [0,17), replicated over 8 nibbles x 4 tiles.
    # eq = [U == iota] gives onehot(U) at k=1..16 and onehot((U+1)%16) at
    # k=0..15 (wrap via the mod-16 table value).
    i17 = ((np.arange(17) + 15) % 16).astype(np.float32)
    iota_rep = np.broadcast_to(i17[None, :, None], (P, 17, 32)).reshape(P, -1)
    ident8 = np.eye(8, dtype=np.float32)
    return (
        W.astype(F8_NP),
        np.ascontiguousarray(iota_rep).astype(BF_NP),
        ident8.astype(BF_NP),
    )


def build_nc():
    nc = bacc.Bacc()
    abT_d = nc.declare_dram_parameter("abT", [FIN, ROWS], F8, isOutput=False)
    wtab_d = nc.declare_dram_parameter("wtab", [P, 8 * KC], F8, isOutput=False)
    iota_d = nc.declare_dram_parameter("iota_rep", [P, 17 * 32], BF, isOutput=False)
    ident_d = nc.declare_dram_parameter("ident8", [8, 8], BF, isOutput=False)
    out_d = nc.declare_dram_parameter("out", [ROWS, FOUT], BF, isOutput=True)

    # input view: [p, c, rows] -> abT[c*128+p, r]; 512B contiguous lines
    abT_v = abT_d[:, :].rearrange("(c p) r -> p c r", p=P)
    # output views: one DMA per dist sub-chunk (4 or 2 row-tiles)
    out4_v = out_d[:, :].rearrange("(q t4 p) f -> q p t4 f", t4=4, p=P)
    out2_v = out_d[:, :].rearrange("(q t2 p) f -> q p t2 f", t2=2, p=P)
    out1_v = out_d[:, :].rearrange("(q p) f -> q p f", p=P)

    AL = mybir.AluOpType

    with TileContext(nc) as tc:
        with (
            tc.tile_pool(name="consts", bufs=1) as cpool,
            tc.tile_pool(name="io", bufs=4) as iopool,
            tc.tile_pool(name="zsbp", bufs=3) as zpool,
            tc.tile_pool(name="carry", bufs=2) as apool,
            tc.tile_pool(name="dist", bufs=3) as dpool,
            tc.tile_pool(name="outp", bufs=3) as opool,
            tc.psum_pool(name="zps", bufs=2) as pzpool,
            tc.psum_pool(name="ztp", bufs=2) as ptpool,
        ):
            wtab = cpool.tile([P, 8 * KC], F8, tag="wtab")
            iota = cpool.tile([P, 17 * 32], BF, tag="iota")
            ident = cpool.tile([8, 8], BF, tag="ident")
            nc.sync.dma_start(wtab[:, :], wtab_d[:, :])
            nc.sync.dma_start(iota[:, :], iota_d[:, :])
            nc.sync.dma_start(ident[:, :], ident_d[:, :])

            # Warm up the PE pipeline during the first input DMA: the PE
            # ramps to full clock only after continuous execution, and the
            # first group's extraction matmuls otherwise run at the slowest
            # pstate on the critical fill path.
            junk = pzpool.tile([8, GR], FP, tag="zps")
            for w in range(14):
                nc.tensor.matmul(
                    junk[:, 0:128],
                    lhsT=wtab[:, 0:8],
                    rhs=wtab[:, 0:128],
                    start=True,
                    stop=True,
                )
            # preload the Act engine's activation-function table off the
            # critical path (the first PSUM evac otherwise pays ~1.3us)
            warmact = zpool.tile([8, GR], BF, tag="zsb")
            nc.scalar.copy(warmact[:, 0:8], ident[:, :])

            # out-DMAs are deferred and flushed on the Act queue after later
            # groups' evacs so they never head-of-line block an evac that
            # the PE transposes (and the whole next chunk) depend on.
            pending_outs = []
            g0 = 0      # first group of current chunk
            t_glob = 0  # first global row-tile of current chunk

            for ch, groups in enumerate(CHUNKS):
                last_ch = ch == len(CHUNKS) - 1
                ntc = sum(groups) // P
                zt = ptpool.tile([P, MAX_NTC * 8], FP, tag="zt")

                t_loc0 = 0  # first tile of this group within the chunk
                for gl, gr_rows in enumerate(groups):
                    r0 = t_glob * P + t_loc0 * P
                    xg = iopool.tile([P, KC * GR], F8, tag="xg")
                    xg_v = xg[:, 0 : KC * gr_rows].rearrange(
                        "p (c r) -> p c r", c=KC
                    )
                    nc.sync.dma_start(xg_v, abT_v[:, :, r0 : r0 + gr_rows])

                    zps = pzpool.tile([8, GR], FP, tag="zps")
                    for c in range(KC):
                        nc.tensor.matmul(
                            zps[:, 0:gr_rows],
                            lhsT=wtab[:, 8 * c : 8 * c + 8],
                            rhs=xg_v[:, c, :],
                            start=(c == 0),
                            stop=(c == KC - 1),
                        )
                    zsb = zpool.tile([8, GR], BF, tag="zsb")
                    nc.scalar.copy(zsb[:, 0:gr_rows], zps[:, 0:gr_rows])
                    for o_view, o4p, sz in pending_outs:
                        nc.scalar.dma_start(o_view, o4p[:, 0 : sz * FOUT])
                    pending_outs = []
                    # PE-transpose the [8, 128] row-blocks back to row-major
                    # [128, 8] via identity matmul (bf16, integer values<=30)
                    for j in range(gr_rows // P):
                        t_loc = t_loc0 + j
                        nc.tensor.matmul(
                            zt[:, 8 * t_loc : 8 * (t_loc + 1)],
                            lhsT=zsb[:, P * j : P * (j + 1)],
                            rhs=ident[:, :],
                            start=True,
                            stop=True,
                        )
                    t_loc0 += gr_rows // P

                # ---- carry chain as ONE prefix scan (reads zt in PSUM) ----
                # Since x is integer and c in [0,1]:
                #   clamp(x+c-15, 0, 1) == max([x>=15.5], min([x>=14.5], c))
                # which is the scan form (data0 min state) max data1.  Per
                # tile: slot 0 is a reset (d0=d1=0.5 forces state=0.5), slots
                # 1..7 use G/P of nibbles 0..6.  Scan output at slot n is
                # then exactly c_in of nibble n.
                NJ = ntc * 8
                zt_tn = zt[:, 0 : NJ].rearrange("p (t n) -> p t n", n=8)
                d0 = apool.tile([P, 8 * MAX_NTC], FP, tag="d0")
                d1 = apool.tile([P, 8 * MAX_NTC], FP, tag="d1")
                c_pre = apool.tile([P, 8 * MAX_NTC], FP, tag="cpre")
                d0_v = d0[:, 0:NJ].rearrange("p (t s) -> p t s", s=8)
                d1_v = d1[:, 0:NJ].rearrange("p (t s) -> p t s", s=8)
                nc.vector.memset(d0[:, 0:NJ], 0.5)
                nc.vector.memset(d1[:, 0:NJ], 0.5)
                nc.vector.tensor_scalar(
                    out=d0_v[:, :, 1:8], in0=zt_tn[:, :, 0:7], scalar1=14.5,
                    scalar2=None, op0=AL.is_ge,
                )
                nc.vector.tensor_scalar(
                    out=d1_v[:, :, 1:8], in0=zt_tn[:, :, 0:7], scalar1=15.5,
                    scalar2=None, op0=AL.is_ge,
                )
                nc.vector.tensor_tensor_scan(
                    out=c_pre[:, 0:NJ], data0=d0[:, 0:NJ], data1=d1[:, 0:NJ],
                    initial=0.5, op0=AL.min, op1=AL.max,
                )

                # ---- vectorized U/P/weights ----
                # flat ops in zt's (t, n)=(t, i, par) layout; u/w0/w1 are
                # written through strided views into (n, t) n-major storage
                # so the dist build gets packed last dims (2x DVE mode).
                p_all = apool.tile([P, 8 * MAX_NTC], FP, tag="pall")
                y_all = apool.tile([P, 8 * MAX_NTC], FP, tag="yall")
                wrap = apool.tile([P, 8 * MAX_NTC], FP, tag="wrap")
                u_all = apool.tile([P, 8 * MAX_NTC], BF, tag="uall")
                w0_all = apool.tile([P, 8 * MAX_NTC], BF, tag="w0")
                w1_all = apool.tile([P, 8 * MAX_NTC], BF, tag="w1")

                def tip(t_ap):  # (t,i,par)-flat tensor -> view [p, t, i, par]
                    return t_ap[:, 0:NJ].rearrange(
                        "p (t i par) -> p t i par", t=ntc, i=4, par=2
                    )

                def pti(t_ap):  # (n,t)=(i,par,t) storage -> [p, t, i, par]
                    return t_ap[:, 0:NJ].rearrange(
                        "p (i par t) -> p t i par", i=4, par=2, t=ntc
                    )

                # y = x + (c_pre >= 0.75), fused into one STT
                nc.vector.scalar_tensor_tensor(
                    out=y_all[:, 0:NJ], in0=c_pre[:, 0:NJ], scalar=0.75,
                    in1=zt[:, 0:NJ], op0=AL.is_ge, op1=AL.add,
                )
                nc.vector.tensor_scalar(
                    out=wrap[:, 0:NJ], in0=y_all[:, 0:NJ], scalar1=15.5,
                    scalar2=None, op0=AL.is_ge,
                )
                nc.vector.scalar_tensor_tensor(
                    out=pti(u_all), in0=tip(wrap), scalar=-16.0,
                    in1=tip(y_all), op0=AL.mult, op1=AL.add,
                )
                nc.vector.tensor_scalar(
                    out=p_all[:, 0:NJ], in0=c_pre[:, 0:NJ], scalar1=0.5,
                    scalar2=None, op0=AL.is_equal,
                )
                nc.vector.tensor_scalar(
                    out=pti(w1_all), in0=tip(p_all), scalar1=0.5,
                    scalar2=None, op0=AL.mult,
                )
                nc.vector.tensor_scalar(
                    out=pti(w0_all), in0=tip(p_all), scalar1=-0.5,
                    scalar2=1.0, op0=AL.mult, op1=AL.add,
                )

                # ---- dist + outers per 4-tile sub (= output quad) ----
                # dist layout (k, n, t): packed t -> 2x DVE mode, and the
                # outer reads keep the measured-fast stride pattern
                # [i:16B, h:64B, l:0].
                u_nv = u_all[:, 0:NJ].rearrange("p (n t) -> p n t", n=8)
                w0_nv = w0_all[:, 0:NJ].rearrange("p (n t) -> p n t", n=8)
                w1_nv = w1_all[:, 0:NJ].rearrange("p (n t) -> p n t", n=8)
                iota_f = iota[:, :].rearrange("p (k n t) -> p k n t", k=17, n=8)
                subs = [(o, min(4, ntc - o)) for o in range(0, ntc, 4)]
                for ts0, st in subs:
                    iota_v = iota_f[:, :, :, 0:st]
                    eqx = dpool.tile([P, 17 * 32], BF, tag="eqx")
                    dsub = dpool.tile([P, 16 * 32], BF, tag="dsub")
                    dtmp = dpool.tile([P, 16 * 32], BF, tag="dtmp")
                    eqx_v = eqx[:, 0 : 17 * 8 * st].rearrange(
                        "p (k n t) -> p k n t", k=17, n=8
                    )
                    dsub_v = dsub[:, 0 : 16 * 8 * st].rearrange(
                        "p (k n t) -> p k n t", k=16, n=8
                    )
                    dtmp_v = dtmp[:, 0 : 16 * 8 * st].rearrange(
                        "p (k n t) -> p k n t", k=16, n=8
                    )
                    u_b = u_nv[:, None, :, ts0 : ts0 + st].broadcast_to([P, 17, 8, st])
                    w0_b = w0_nv[:, None, :, ts0 : ts0 + st].broadcast_to([P, 16, 8, st])
                    w1_b = w1_nv[:, None, :, ts0 : ts0 + st].broadcast_to([P, 16, 8, st])
                    nc.vector.tensor_tensor(eqx_v, u_b, iota_v, op=AL.is_equal)
                    nc.vector.tensor_mul(dsub_v, eqx_v[:, 1:17], w0_b)
                    nc.vector.tensor_mul(dtmp_v, eqx_v[:, 0:16], w1_b)
                    nc.vector.tensor_add(
                        dsub[:, 0 : 16 * 8 * st],
                        dsub[:, 0 : 16 * 8 * st],
                        dtmp[:, 0 : 16 * 8 * st],
                    )

                    # dv: [p, k(nibble value), i(byte), par(lo/hi), t]
                    dv = dsub[:, 0 : 16 * 8 * st].rearrange(
                        "p (k i par t) -> p k i par t", k=16, i=4, par=2, t=st
                    )
                    # Outers: a broadcast-AP TT measures ~1.85 ns/elem on DVE
                    # regardless of dtype, while copies/TS with arbitrary
                    # strides run in the 2x_2p mode (~0.55) and packed bf16
                    # TTs in 2x_1p.  So materialize the replicated factors --
                    # h_rep on the mostly-idle Act engine, l_rep on DVE --
                    # and do one all-packed bf16 multiply per tile.
                    o4 = opool.tile([P, 4 * FOUT], BF, tag="o4")
                    hrep = opool.tile([P, 4 * FOUT], BF, tag="hrep")
                    lrep = opool.tile([P, 4 * FOUT], BF, tag="lrep")
                    for tl in range(st):
                        h_t = (
                            dv[:, :, :, 1, tl]
                            .rearrange("p k i -> p i k")[:, :, :, None]
                            .broadcast_to([P, 4, 16, 16])
                        )
                        l_t = (
                            dv[:, :, :, 0, tl]
                            .rearrange("p k i -> p i k")[:, :, None, :]
                            .broadcast_to([P, 4, 16, 16])
                        )
                        hrep_v = hrep[:, tl * FOUT : (tl + 1) * FOUT].rearrange(
                            "p (i h l) -> p i h l", h=16, l=16
                        )
                        lrep_v = lrep[:, tl * FOUT : (tl + 1) * FOUT].rearrange(
                            "p (i h l) -> p i h l", h=16, l=16
                        )
                        nc.scalar.copy(hrep_v, h_t)
                        nc.vector.tensor_copy(lrep_v, l_t)
                    for m0 in range(0, st, 2):
                        mw = min(2, st - m0)
                        nc.vector.tensor_mul(
                            o4[:, m0 * FOUT : (m0 + mw) * FOUT],
                            hrep[:, m0 * FOUT : (m0 + mw) * FOUT],
                            lrep[:, m0 * FOUT : (m0 + mw) * FOUT],
                        )
                    t_first = t_glob + ts0
                    if st == 4:
                        o_view = out4_v[t_first // 4]
                    elif st == 2:
                        o_view = out2_v[t_first // 2]
                    else:
                        o_view = out1_v[t_first]
                    if last_ch:
                        nc.scalar.dma_start(o_view, o4[:, 0 : st * FOUT])
                    else:
                        pending_outs.append((o_view, o4, st))

                t_glob += ntc

            for o_view, o4p, sz in pending_outs:
                nc.scalar.dma_start(o_view, o4p[:, 0 : sz * FOUT])

    nc.finalize()
    return nc


_NC_CACHE = {}
LAST_RESULT = None


def kernel(**inputs) -> np.ndarray:
    global LAST_RESULT
    a = np.asarray(inputs["a"], dtype=np.float32).reshape(B_FULL, 1024)
    b = np.asarray(inputs["b"], dtype=np.float32).reshape(B_FULL, 1024)
    ab = np.concatenate([a, b], axis=1).astype(F8_NP)  # [B, 2048] fp8, exact
    wtab, iota_rep, ident8 = _const_tables()

    if "nc" not in _NC_CACHE:
        _NC_CACHE["nc"] = build_nc()
    nc = _NC_CACHE["nc"]

    in_maps = []
    for c in range(N_CORES):
        abT = np.ascontiguousarray(ab[c * ROWS : (c + 1) * ROWS].T)  # [2048, 4096]
        in_maps.append({
            "abT": abT,
            "wtab": wtab,
            "iota_rep": iota_rep,
            "ident8": ident8,
        })
    res = run_bass_kernel_spmd(nc, in_maps, core_ids=list(range(N_CORES)))
    LAST_RESULT = res
    out = np.concatenate([r["out"] for r in res.results], axis=0)  # bf16
    return out.astype(np.float32).reshape(B_FULL, 4, 256)


# revision 48
# speedup vs baseline: 1.0495x; 1.0495x over previous
"""Trainium2 Bass kernel for nn_NeuralALU (batched byte-encoded 32-bit add).

The reference network computes, per batch element, a chain of table-lookup
matmuls + sharp softmaxes (scale=100) over exactly-one-hot byte encodings.
Because the inputs are exact one-hots, the float pipeline collapses to a
discrete algorithm (validated to ~1e-22 rel-err):

  xl = (a%16 + b%16), xh = (a>>4 + b>>4)           per byte, in [0,30]
  carry state c in {0, 0.5, 1}, init 0.5, over 8 nibbles (lo0,hi0,...,hi3):
      add = (c == 1); y = x + add; U = y mod 16; P = (c == 0.5)
      c' = clamp(x + c - 15, 0, 1)
  nibble dist = onehot(U)*(1-P/2) + onehot((U+1) mod 16)*(P/2)
  out byte row [256] = outer(h_dist, l_dist) flattened

Final architecture (230us baseline -> ~104us):
  - Input staged host-side as fp8-e4m3 (one-hots {0,1} are exact) and
    TRANSPOSED to [2048, 4096] per core: 8 MiB/core input traffic (vs 32
    for f32 row-major), read in [128, c, rows] tiles with 512B lines.
  - Nibble-sum extraction on the otherwise-idle TensorEngine: stationary
    [128, 8] code tables (lo/hi nibble value per one-hot position), moving
    [128, rows] input columns, f32 PSUM accumulation over 16 k-chunks.
    The a- and b-halves sum in the same accumulation, so PSUM IS xlo/xhi
    per byte -- no DVE dot-products and no i32 bit-extraction at all.
    A burst of dummy matmuls under the first input DMA warms the PE
    pstate off the critical fill path.
  - [8, rows] PSUM results cast to bf16 (Act engine) and PE-transposed to
    row-major [128, 8] via tiny identity matmuls.
  - The sequential carry chain is ONE tensor_tensor_scan per chunk: since
    x is integer and c in [0,1], clamp(x+c-15,0,1) == max([x>=15.5],
    min([x>=14.5], c)), which is the scan form (d0 min state) max d1 with
    per-tile reset slots (d0=d1=0.5).
  - Chunked processing with small chunks at both ends ([2,2,4,8,8,4,2,2]
    row-tiles) to cut pipeline-fill latency and the serial tail.
  - DVE dist build in bf16, (k, nibble, tile) layout: packed last dims
    keep it in the 2x DVE mode.
  - Outer products: broadcast-AP tensor ops measure ~1.85 ns/elem on DVE
    regardless of dtype, while arbitrary-stride READS through TensorCopy/
    TensorScalar run in the 2x_2p mode and packed bf16 TTs in 2x_1p
    (strided WRITES are the real poison -- keep output APs packed).  So:
    h_rep is materialized on the Act engine, l_rep via a DVE copy, and
    the outer is one all-packed bf16 multiply per tile pair.
  - Output written bf16 (values in {0,.25,.5,1}, exact), upcast host-side:
    8 MiB/core output traffic.
  - Input DMAs ride the SP HWDGE queue; PSUM evacs, h_reps and output
    DMAs the Act queue, with output issues deferred behind the next
    group's evac so they never head-of-line block the PE transposes.

Engine busy per core (of ~104us wall): DVE ~73, PE ~68, Act ~60,
DMA ~58.  Sharding: pure data parallel over batch across 8 NeuronCores.
"""

import numpy as np
import ml_dtypes

import concourse.bass as bass
import concourse.bacc as bacc
import concourse.mybir as mybir
from concourse.tile import TileContext
from concourse.bass_utils import run_bass_kernel_spmd

N_CORES = 8
B_FULL = 32768
ROWS = B_FULL // N_CORES   # 4096 rows per core
P = 128
FIN = 2048                 # a|b one-hot columns, concatenated
KC = FIN // P              # 16 k-chunks
GR = 512                   # rows per matmul group (max moving free dim)
NG = ROWS // GR            # 8 groups
TPG = GR // P              # 4 row-tiles per group
# chunks as lists of group row-counts: small chunks at the ends cut the
# pipeline-fill latency (first zt ready sooner) and the serial tail
# (last quad DMA is smaller); 512-row groups in the middle for efficiency
CHUNKS = [[256], [256], [512], [512, 512], [512, 512], [512], [256], [256]]
FOUT = 1024                # 4 bytes x 256 output row

FP = mybir.dt.float32
BF = mybir.dt.bfloat16
F8 = mybir.dt.float8e4
BF_NP = ml_dtypes.bfloat16
F8_NP = ml_dtypes.float8_e4m3

MAX_NTC = max(sum(g) for g in CHUNKS) // P
assert sum(sum(g) for g in CHUNKS) == ROWS


def _const_tables():
    # Code table: for k-chunk c, wtab[p, 8c + 2i + s] = nibble value (lo if
    # s=0 else hi) of one-hot position f = 128c+p, where i is the byte
    # index of f within its a/b half.  Values 0..15: fp8-e4m3-exact.
    W = np.zeros((P, 8 * KC), np.float32)
    f = np.arange(FIN)
    fb = f % 1024
    i_b = fb // 256
    k = fb % 256
    c = f // P
    p = f % P
    W[p, 8 * c + 2 * i_b] = k & 15
    W[p, 8 * c + 2 * i_b + 1] = k >> 4
    # Padded compare table, layout (k, n, t): iota_rep[p, (k,n,t)] =
    # (k-1) mod 16 for k in [0,17), replicated over 8 nibbles x 4 tiles.
    # eq = [U == iota] gives onehot(U) at k=1..16 and onehot((U+1)%16) at
    # k=0..15 (wrap via the mod-16 table value).
    i17 = ((np.arange(17) + 15) % 16).astype(np.float32)
    iota_rep = np.broadcast_to(i17[None, :, None], (P, 17, 32)).reshape(P, -1)
    ident8 = np.eye(8, dtype=np.float32)
    return (
        W.astype(F8_NP),
        np.ascontiguousarray(iota_rep).astype(BF_NP),
        ident8.astype(BF_NP),
    )


def build_nc():
    nc = bacc.Bacc()
    abT_d = nc.declare_dram_parameter("abT", [FIN, ROWS], F8, isOutput=False)
    wtab_d = nc.declare_dram_parameter("wtab", [P, 8 * KC], F8, isOutput=False)
    iota_d = nc.declare_dram_parameter("iota_rep", [P, 17 * 32], BF, isOutput=False)
    ident_d = nc.declare_dram_parameter("ident8", [8, 8], BF, isOutput=False)
    out_d = nc.declare_dram_parameter("out", [ROWS, FOUT], BF, isOutput=True)

    # input view: [p, c, rows] -> abT[c*128+p, r]; 512B contiguous lines
    abT_v = abT_d[:, :].rearrange("(c p) r -> p c r", p=P)
    # output views: one DMA per dist sub-chunk (4 or 2 row-tiles)
    out4_v = out_d[:, :].rearrange("(q t4 p) f -> q p t4 f", t4=4, p=P)
    out2_v = out_d[:, :].rearrange("(q t2 p) f -> q p t2 f", t2=2, p=P)
    out1_v = out_d[:, :].rearrange("(q p) f -> q p f", p=P)

    AL = mybir.AluOpType

    with TileContext(nc) as tc:
        with (
            tc.tile_pool(name="consts", bufs=1) as cpool,
            tc.tile_pool(name="io", bufs=4) as iopool,
            tc.tile_pool(name="zsbp", bufs=3) as zpool,
            tc.tile_pool(name="carry", bufs=2) as apool,
            tc.tile_pool(name="dist", bufs=3) as dpool,
            tc.tile_pool(name="outp", bufs=3) as opool,
            tc.psum_pool(name="zps", bufs=2) as pzpool,
            tc.psum_pool(name="ztp", bufs=2) as ptpool,
        ):
            wtab = cpool.tile([P, 8 * KC], F8, tag="wtab")
            iota = cpool.tile([P, 17 * 32], BF, tag="iota")
            ident = cpool.tile([8, 8], BF, tag="ident")
            nc.sync.dma_start(wtab[:, :], wtab_d[:, :])
            nc.sync.dma_start(iota[:, :], iota_d[:, :])
            nc.sync.dma_start(ident[:, :], ident_d[:, :])

            # Warm up the PE pipeline during the first input DMA: the PE
            # ramps to full clock only after continuous execution, and the
            # first group's extraction matmuls otherwise run at the slowest
            # pstate on the critical fill path.
            junk = pzpool.tile([8, GR], FP, tag="zps")
            for w in range(14):
                nc.tensor.matmul(
                    junk[:, 0:128],
                    lhsT=wtab[:, 0:8],
                    rhs=wtab[:, 0:128],
                    start=True,
                    stop=True,
                )
            # preload the Act engine's activation-function table off the
            # critical path (the first PSUM evac otherwise pays ~1.3us)
            warmact = zpool.tile([8, GR], BF, tag="zsb")
            nc.scalar.copy(warmact[:, 0:8], ident[:, :])

            # out-DMAs are deferred and flushed on the Act queue after later
            # groups' evacs so they never head-of-line block an evac that
            # the PE transposes (and the whole next chunk) depend on.
            pending_outs = []
            t_glob = 0  # first global row-tile of current chunk

            for ch, groups in enumerate(CHUNKS):
                last_ch = ch == len(CHUNKS) - 1
                ntc = sum(groups) // P
                zt = ptpool.tile([P, MAX_NTC * 8], FP, tag="zt")

                t_loc0 = 0  # first tile of this group within the chunk
                for gl, gr_rows in enumerate(groups):
                    r0 = t_glob * P + t_loc0 * P
                    xg = iopool.tile([P, KC * GR], F8, tag="xg")
                    xg_v = xg[:, 0 : KC * gr_rows].rearrange(
                        "p (c r) -> p c r", c=KC
                    )
                    nc.sync.dma_start(xg_v, abT_v[:, :, r0 : r0 + gr_rows])

                    zps = pzpool.tile([8, GR], FP, tag="zps")
                    for c in range(KC):
                        nc.tensor.matmul(
                            zps[:, 0:gr_rows],
                            lhsT=wtab[:, 8 * c : 8 * c + 8],
                            rhs=xg_v[:, c, :],
                            start=(c == 0),
                            stop=(c == KC - 1),
                        )
                    zsb = zpool.tile([8, GR], BF, tag="zsb")
                    nc.scalar.copy(zsb[:, 0:gr_rows], zps[:, 0:gr_rows])
                    for o_view, o4p, sz in pending_outs:
                        nc.scalar.dma_start(o_view, o4p[:, 0 : sz * FOUT])
                    pending_outs = []
                    # PE-transpose the [8, 128] row-blocks back to row-major
                    # [128, 8] via identity matmul (bf16, integer values<=30)
                    for j in range(gr_rows // P):
                        t_loc = t_loc0 + j
                        nc.tensor.matmul(
                            zt[:, 8 * t_loc : 8 * (t_loc + 1)],
                            lhsT=zsb[:, P * j : P * (j + 1)],
                            rhs=ident[:, :],
                            start=True,
                            stop=True,
                        )
                    t_loc0 += gr_rows // P

                # ---- carry chain as ONE prefix scan (reads zt in PSUM) ----
                # Since x is integer and c in [0,1]:
                #   clamp(x+c-15, 0, 1) == max([x>=15.5], min([x>=14.5], c))
                # which is the scan form (data0 min state) max data1.  Per
                # tile: slot 0 is a reset (d0=d1=0.5 forces state=0.5), slots
                # 1..7 use G/P of nibbles 0..6.  Scan output at slot n is
                # then exactly c_in of nibble n.
                NJ = ntc * 8
                zt_tn = zt[:, 0 : NJ].rearrange("p (t n) -> p t n", n=8)
                d0 = apool.tile([P, 8 * MAX_NTC], FP, tag="d0")
                d1 = apool.tile([P, 8 * MAX_NTC], FP, tag="d1")
                c_pre = apool.tile([P, 8 * MAX_NTC], FP, tag="cpre")
                d0_v = d0[:, 0:NJ].rearrange("p (t s) -> p t s", s=8)
                d1_v = d1[:, 0:NJ].rearrange("p (t s) -> p t s", s=8)
                nc.vector.memset(d0[:, 0:NJ], 0.5)
                nc.vector.memset(d1[:, 0:NJ], 0.5)
                nc.vector.tensor_scalar(
                    out=d0_v[:, :, 1:8], in0=zt_tn[:, :, 0:7], scalar1=14.5,
                    scalar2=None, op0=AL.is_ge,
                )
                nc.vector.tensor_scalar(
                    out=d1_v[:, :, 1:8], in0=zt_tn[:, :, 0:7], scalar1=15.5,
                    scalar2=None, op0=AL.is_ge,
                )
                nc.vector.tensor_tensor_scan(
                    out=c_pre[:, 0:NJ], data0=d0[:, 0:NJ], data1=d1[:, 0:NJ],
                    initial=0.5, op0=AL.min, op1=AL.max,
                )

                # ---- vectorized U/P/weights ----
                # flat ops in zt's (t, n)=(t, i, par) layout; u/w0/w1 are
                # written through strided views into (n, t) n-major storage
                # so the dist build gets packed last dims (2x DVE mode).
                p_all = apool.tile([P, 8 * MAX_NTC], FP, tag="pall")
                y_all = apool.tile([P, 8 * MAX_NTC], FP, tag="yall")
                wrap = apool.tile([P, 8 * MAX_NTC], FP, tag="wrap")
                u_all = apool.tile([P, 8 * MAX_NTC], BF, tag="uall")
                w0_all = apool.tile([P, 8 * MAX_NTC], BF, tag="w0")
                w1_all = apool.tile([P, 8 * MAX_NTC], BF, tag="w1")

                def tip(t_ap):  # (t,i,par)-flat tensor -> view [p, t, i, par]
                    return t_ap[:, 0:NJ].rearrange(
                        "p (t i par) -> p t i par", t=ntc, i=4, par=2
                    )

                def pti(t_ap):  # (n,t)=(i,par,t) storage -> [p, t, i, par]
                    return t_ap[:, 0:NJ].rearrange(
                        "p (i par t) -> p t i par", i=4, par=2, t=ntc
                    )

                # y = x + (c_pre >= 0.75), fused into one STT
                nc.vector.scalar_tensor_tensor(
                    out=y_all[:, 0:NJ], in0=c_pre[:, 0:NJ], scalar=0.75,
                    in1=zt[:, 0:NJ], op0=AL.is_ge, op1=AL.add,
                )
                nc.vector.tensor_scalar(
                    out=wrap[:, 0:NJ], in0=y_all[:, 0:NJ], scalar1=15.5,
                    scalar2=None, op0=AL.is_ge,
                )
                nc.vector.scalar_tensor_tensor(
                    out=pti(u_all), in0=tip(wrap), scalar=-16.0,
                    in1=tip(y_all), op0=AL.mult, op1=AL.add,
                )
                nc.vector.tensor_scalar(
                    out=p_all[:, 0:NJ], in0=c_pre[:, 0:NJ], scalar1=0.5,
                    scalar2=None, op0=AL.is_equal,
                )
                nc.vector.tensor_scalar(
                    out=pti(w1_all), in0=tip(p_all), scalar1=0.5,
                    scalar2=None, op0=AL.mult,
                )
                nc.vector.tensor_scalar(
                    out=pti(w0_all), in0=tip(p_all), scalar1=-0.5,
                    scalar2=1.0, op0=AL.mult, op1=AL.add,
                )

                # ---- dist + outers per 4-tile sub (= output quad) ----
                # dist layout (k, n, t): packed t -> 2x DVE mode, and the
                # outer reads keep the measured-fast stride pattern
                # [i:16B, h:64B, l:0].
                u_nv = u_all[:, 0:NJ].rearrange("p (n t) -> p n t", n=8)
                w0_nv = w0_all[:, 0:NJ].rearrange("p (n t) -> p n t", n=8)
                w1_nv = w1_all[:, 0:NJ].rearrange("p (n t) -> p n t", n=8)
                iota_f = iota[:, :].rearrange("p (k n t) -> p k n t", k=17, n=8)
                subs = [(o, min(4, ntc - o)) for o in range(0, ntc, 4)]
                for ts0, st in subs:
                    iota_v = iota_f[:, :, :, 0:st]
                    eqx = dpool.tile([P, 17 * 32], BF, tag="eqx")
                    dsub = dpool.tile([P, 16 * 32], BF, tag="dsub")
                    dtmp = dpool.tile([P, 16 * 32], BF, tag="dtmp")
                    eqx_v = eqx[:, 0 : 17 * 8 * st].rearrange(
                        "p (k n t) -> p k n t", k=17, n=8
                    )
                    dsub_v = dsub[:, 0 : 16 * 8 * st].rearrange(
                        "p (k n t) -> p k n t", k=16, n=8
                    )
                    dtmp_v = dtmp[:, 0 : 16 * 8 * st].rearrange(
                        "p (k n t) -> p k n t", k=16, n=8
                    )
                    u_b = u_nv[:, None, :, ts0 : ts0 + st].broadcast_to([P, 17, 8, st])
                    w0_b = w0_nv[:, None, :, ts0 : ts0 + st].broadcast_to([P, 16, 8, st])
                    w1_b = w1_nv[:, None, :, ts0 : ts0 + st].broadcast_to([P, 16, 8, st])
                    nc.vector.tensor_tensor(eqx_v, u_b, iota_v, op=AL.is_equal)
                    nc.vector.tensor_mul(dsub_v, eqx_v[:, 1:17], w0_b)
                    nc.vector.tensor_mul(dtmp_v, eqx_v[:, 0:16], w1_b)
                    nc.vector.tensor_add(
                        dsub[:, 0 : 16 * 8 * st],
                        dsub[:, 0 : 16 * 8 * st],
                        dtmp[:, 0 : 16 * 8 * st],
                    )

                    # dv: [p, k(nibble value), i(byte), par(lo/hi), t]
                    dv = dsub[:, 0 : 16 * 8 * st].rearrange(
                        "p (k i par t) -> p k i par t", k=16, i=4, par=2, t=st
                    )
                    # Outers: a broadcast-AP TT measures ~1.85 ns/elem on DVE
                    # regardless of dtype, while copies/TS with arbitrary
                    # strides run in the 2x_2p mode (~0.55) and packed bf16
                    # TTs in 2x_1p.  So materialize the replicated factors --
                    # h_rep on the mostly-idle Act engine, l_rep on DVE --
                    # and do one all-packed bf16 multiply per tile.
                    o4 = opool.tile([P, 4 * FOUT], BF, tag="o4")
                    hrep = opool.tile([P, 4 * FOUT], BF, tag="hrep")
                    lrep = opool.tile([P, 4 * FOUT], BF, tag="lrep")
                    for tl in range(st):
                        h_t = (
                            dv[:, :, :, 1, tl]
                            .rearrange("p k i -> p i k")[:, :, :, None]
                            .broadcast_to([P, 4, 16, 16])
                        )
                        l_t = (
                            dv[:, :, :, 0, tl]
                            .rearrange("p k i -> p i k")[:, :, None, :]
                            .broadcast_to([P, 4, 16, 16])
                        )
                        hrep_v = hrep[:, tl * FOUT : (tl + 1) * FOUT].rearrange(
                            "p (i h l) -> p i h l", h=16, l=16
                        )
                        lrep_v = lrep[:, tl * FOUT : (tl + 1) * FOUT].rearrange(
                            "p (i h l) -> p i h l", h=16, l=16
                        )
                        nc.scalar.copy(hrep_v, h_t)
                        nc.vector.tensor_copy(lrep_v, l_t)
                    for m0 in range(0, st, 2):
                        mw = min(2, st - m0)
                        nc.vector.tensor_mul(
                            o4[:, m0 * FOUT : (m0 + mw) * FOUT],
                            hrep[:, m0 * FOUT : (m0 + mw) * FOUT],
                            lrep[:, m0 * FOUT : (m0 + mw) * FOUT],
                        )
                    t_first = t_glob + ts0
                    if st == 4:
                        o_view = out4_v[t_first // 4]
                    elif st == 2:
                        o_view = out2_v[t_first // 2]
                    else:
                        o_view = out1_v[t_first]
                    if last_ch:
                        nc.scalar.dma_start(o_view, o4[:, 0 : st * FOUT])
                    else:
                        pending_outs.append((o_view, o4, st))

                t_glob += ntc

            for o_view, o4p, sz in pending_outs:
                nc.scalar.dma_start(o_view, o4p[:, 0 : sz * FOUT])

    nc.finalize()
    return nc


_NC_CACHE = {}
LAST_RESULT = None


def kernel(**inputs) -> np.ndarray:
    global LAST_RESULT
    a = np.asarray(inputs["a"], dtype=np.float32).reshape(B_FULL, 1024)
    b = np.asarray(inputs["b"], dtype=np.float32).reshape(B_FULL, 1024)
    ab = np.concatenate([a, b], axis=1).astype(F8_NP)  # [B, 2048] fp8, exact
    wtab, iota_rep, ident8 = _const_tables()

    if "nc" not in _NC_CACHE:
        _NC_CACHE["nc"] = build_nc()
    nc = _NC_CACHE["nc"]

    in_maps = []
    for c in range(N_CORES):
        abT = np.ascontiguousarray(ab[c * ROWS : (c + 1) * ROWS].T)  # [2048, 4096]
        in_maps.append({
            "abT": abT,
            "wtab": wtab,
            "iota_rep": iota_rep,
            "ident8": ident8,
        })
    res = run_bass_kernel_spmd(nc, in_maps, core_ids=list(range(N_CORES)))
    LAST_RESULT = res
    out = np.concatenate([r["out"] for r in res.results], axis=0)  # bf16
    return out.astype(np.float32).reshape(B_FULL, 4, 256)


# revision 50
# speedup vs baseline: 1.0868x; 1.0355x over previous
"""Trainium2 Bass kernel for nn_NeuralALU (batched byte-encoded 32-bit add).

The reference network computes, per batch element, a chain of table-lookup
matmuls + sharp softmaxes (scale=100) over exactly-one-hot byte encodings.
Because the inputs are exact one-hots, the float pipeline collapses to a
discrete algorithm (validated to ~1e-22 rel-err):

  xl = (a%16 + b%16), xh = (a>>4 + b>>4)           per byte, in [0,30]
  carry state c in {0, 0.5, 1}, init 0.5, over 8 nibbles (lo0,hi0,...,hi3):
      add = (c == 1); y = x + add; U = y mod 16; P = (c == 0.5)
      c' = clamp(x + c - 15, 0, 1)
  nibble dist = onehot(U)*(1-P/2) + onehot((U+1) mod 16)*(P/2)
  out byte row [256] = outer(h_dist, l_dist) flattened

Final architecture (230us baseline -> ~104us):
  - Input staged host-side as fp8-e4m3 (one-hots {0,1} are exact) and
    TRANSPOSED to [2048, 4096] per core: 8 MiB/core input traffic (vs 32
    for f32 row-major), read in [128, c, rows] tiles with 512B lines.
  - Nibble-sum extraction on the otherwise-idle TensorEngine: stationary
    [128, 8] code tables (lo/hi nibble value per one-hot position), moving
    [128, rows] input columns, f32 PSUM accumulation over 16 k-chunks.
    The a- and b-halves sum in the same accumulation, so PSUM IS xlo/xhi
    per byte -- no DVE dot-products and no i32 bit-extraction at all.
    A burst of dummy matmuls under the first input DMA warms the PE
    pstate off the critical fill path.
  - [8, rows] PSUM results cast to bf16 (Act engine) and PE-transposed to
    row-major [128, 8] via tiny identity matmuls.
  - The sequential carry chain is ONE tensor_tensor_scan per chunk: since
    x is integer and c in [0,1], clamp(x+c-15,0,1) == max([x>=15.5],
    min([x>=14.5], c)), which is the scan form (d0 min state) max d1 with
    per-tile reset slots (d0=d1=0.5).
  - Chunked processing with small chunks at both ends ([2,2,4,8,8,4,2,2]
    row-tiles) to cut pipeline-fill latency and the serial tail.
  - DVE dist build in bf16, (k, nibble, tile) layout: packed last dims
    keep it in the 2x DVE mode.
  - Outer products: broadcast-AP tensor ops measure ~1.85 ns/elem on DVE
    regardless of dtype, while arbitrary-stride READS through TensorCopy/
    TensorScalar run in the 2x_2p mode and packed bf16 TTs in 2x_1p
    (strided WRITES are the real poison -- keep output APs packed).  So:
    h_rep is materialized on the Act engine, l_rep via a DVE copy, and
    the outer is one all-packed bf16 multiply per tile pair.
  - Output written bf16 (values in {0,.25,.5,1}, exact), upcast host-side:
    8 MiB/core output traffic.
  - Input DMAs ride the SP HWDGE queue; PSUM evacs, h_reps and output
    DMAs the Act queue, with output issues deferred behind the next
    group's evac so they never head-of-line block the PE transposes.

Engine busy per core (of ~104us wall): DVE ~73, PE ~68, Act ~60,
DMA ~58.  Sharding: pure data parallel over batch across 8 NeuronCores.
"""

import numpy as np
import ml_dtypes

import concourse.bass as bass
import concourse.bacc as bacc
import concourse.mybir as mybir
from concourse.tile import TileContext
from concourse.bass_utils import run_bass_kernel_spmd

N_CORES = 8
B_FULL = 32768
ROWS = B_FULL // N_CORES   # 4096 rows per core
P = 128
FIN = 2048                 # a|b one-hot columns, concatenated
KC = FIN // P              # 16 k-chunks
GR = 512                   # rows per matmul group (max moving free dim)
NG = ROWS // GR            # 8 groups
TPG = GR // P              # 4 row-tiles per group
# chunks as lists of group row-counts: small chunks at the ends cut the
# pipeline-fill latency (first zt ready sooner) and the serial tail
# (last quad DMA is smaller); 512-row groups in the middle for efficiency
CHUNKS = [[256], [256], [512], [512, 512], [512, 512], [512], [256], [256]]
FOUT = 1024                # 4 bytes x 256 output row

FP = mybir.dt.float32
BF = mybir.dt.bfloat16
F8 = mybir.dt.float8e4
BF_NP = ml_dtypes.bfloat16
F8_NP = ml_dtypes.float8_e4m3

MAX_NTC = max(sum(g) for g in CHUNKS) // P
assert sum(sum(g) for g in CHUNKS) == ROWS


def _const_tables():
    # Code table: for k-chunk c, wtab[p, 8c + 2i + s] = nibble value (lo if
    # s=0 else hi) of one-hot position f = 128c+p, where i is the byte
    # index of f within its a/b half.  Values 0..15: fp8-e4m3-exact.
    W = np.zeros((P, 8 * KC), np.float32)
    f = np.arange(FIN)
    fb = f % 1024
    i_b = fb // 256
    k = fb % 256
    c = f // P
    p = f % P
    W[p, 8 * c + 2 * i_b] = k & 15
    W[p, 8 * c + 2 * i_b + 1] = k >> 4
    # Padded compare table, layout (k, n, t): iota_rep[p, (k,n,t)] =
    # (k-1) mod 16 for k in [0,17), replicated over 8 nibbles x 4 tiles.
    # eq = [U == iota] gives onehot(U) at k=1..16 and onehot((U+1)%16) at
    # k=0..15 (wrap via the mod-16 table value).
    i17 = ((np.arange(17) + 15) % 16).astype(np.float32)
    iota_rep = np.broadcast_to(i17[None, :, None], (P, 17, 32)).reshape(P, -1)
    ident8 = np.eye(8, dtype=np.float32)
    return (
        W.astype(F8_NP),
        np.ascontiguousarray(iota_rep).astype(BF_NP),
        ident8.astype(BF_NP),
    )


def build_nc():
    nc = bacc.Bacc()
    abT_d = nc.declare_dram_parameter("abT", [FIN, ROWS], F8, isOutput=False)
    wtab_d = nc.declare_dram_parameter("wtab", [P, 8 * KC], F8, isOutput=False)
    iota_d = nc.declare_dram_parameter("iota_rep", [P, 17 * 32], BF, isOutput=False)
    ident_d = nc.declare_dram_parameter("ident8", [8, 8], BF, isOutput=False)
    out_d = nc.declare_dram_parameter("out", [ROWS, FOUT], BF, isOutput=True)

    # input view: [p, c, rows] -> abT[c*128+p, r]; 512B contiguous lines
    abT_v = abT_d[:, :].rearrange("(c p) r -> p c r", p=P)
    # output views: one DMA per dist sub-chunk (4 or 2 row-tiles)
    out4_v = out_d[:, :].rearrange("(q t4 p) f -> q p t4 f", t4=4, p=P)
    out2_v = out_d[:, :].rearrange("(q t2 p) f -> q p t2 f", t2=2, p=P)
    out1_v = out_d[:, :].rearrange("(q p) f -> q p f", p=P)

    AL = mybir.AluOpType

    with TileContext(nc) as tc:
        with (
            tc.tile_pool(name="consts", bufs=1) as cpool,
            tc.tile_pool(name="io", bufs=4) as iopool,
            tc.tile_pool(name="zsbp", bufs=3) as zpool,
            tc.tile_pool(name="carry", bufs=2) as apool,
            tc.tile_pool(name="dist", bufs=3) as dpool,
            tc.tile_pool(name="outp", bufs=3) as opool,
            tc.psum_pool(name="zps", bufs=2) as pzpool,
            tc.psum_pool(name="ztp", bufs=2) as ptpool,
        ):
            wtab = cpool.tile([P, 8 * KC], F8, tag="wtab")
            iota = cpool.tile([P, 17 * 32], BF, tag="iota")
            ident = cpool.tile([8, 8], BF, tag="ident")
            nc.sync.dma_start(wtab[:, :], wtab_d[:, :])
            nc.sync.dma_start(iota[:, :], iota_d[:, :])
            nc.sync.dma_start(ident[:, :], ident_d[:, :])

            # Warm up the PE pipeline during the first input DMA: the PE
            # ramps to full clock only after continuous execution, and the
            # first group's extraction matmuls otherwise run at the slowest
            # pstate on the critical fill path.
            junk = pzpool.tile([8, GR], FP, tag="zps")
            for w in range(14):
                nc.tensor.matmul(
                    junk[:, 0:128],
                    lhsT=wtab[:, 0:8],
                    rhs=wtab[:, 0:128],
                    start=True,
                    stop=True,
                )
            # preload the Act engine's activation-function table off the
            # critical path (the first PSUM evac otherwise pays ~1.3us)
            warmact = zpool.tile([8, GR], BF, tag="zsb")
            nc.scalar.copy(warmact[:, 0:8], ident[:, :])

            # out-DMAs are deferred and flushed on the Act queue after later
            # groups' evacs so they never head-of-line block an evac that
            # the PE transposes (and the whole next chunk) depend on.
            pending_outs = []
            t_glob = 0  # first global row-tile of current chunk

            for ch, groups in enumerate(CHUNKS):
                last_ch = ch == len(CHUNKS) - 1
                ntc = sum(groups) // P
                zt = ptpool.tile([P, MAX_NTC * 8], FP, tag="zt")

                t_loc0 = 0  # first tile of this group within the chunk
                for gl, gr_rows in enumerate(groups):
                    r0 = t_glob * P + t_loc0 * P
                    xg = iopool.tile([P, KC * GR], F8, tag="xg")
                    xg_v = xg[:, 0 : KC * gr_rows].rearrange(
                        "p (c r) -> p c r", c=KC
                    )
                    nc.sync.dma_start(xg_v, abT_v[:, :, r0 : r0 + gr_rows])

                    zps = pzpool.tile([8, GR], FP, tag="zps")
                    for c in range(KC):
                        nc.tensor.matmul(
                            zps[:, 0:gr_rows],
                            lhsT=wtab[:, 8 * c : 8 * c + 8],
                            rhs=xg_v[:, c, :],
                            start=(c == 0),
                            stop=(c == KC - 1),
                        )
                    zsb = zpool.tile([8, GR], BF, tag="zsb")
                    nc.scalar.copy(zsb[:, 0:gr_rows], zps[:, 0:gr_rows])
                    for o_view, o4p, sz in pending_outs:
                        nc.scalar.dma_start(o_view, o4p[:, 0 : sz * FOUT])
                    pending_outs = []
                    # PE-transpose the [8, 128] row-blocks back to row-major
                    # [128, 8] via identity matmul (bf16, integer values<=30)
                    for j in range(gr_rows // P):
                        t_loc = t_loc0 + j
                        nc.tensor.matmul(
                            zt[:, 8 * t_loc : 8 * (t_loc + 1)],
                            lhsT=zsb[:, P * j : P * (j + 1)],
                            rhs=ident[:, :],
                            start=True,
                            stop=True,
                        )
                    t_loc0 += gr_rows // P

                # ---- carry chain as ONE prefix scan (reads zt in PSUM) ----
                # Since x is integer and c in [0,1]:
                #   clamp(x+c-15, 0, 1) == max([x>=15.5], min([x>=14.5], c))
                # which is the scan form (data0 min state) max data1.  Per
                # tile: slot 0 is a reset (d0=d1=0.5 forces state=0.5), slots
                # 1..7 use G/P of nibbles 0..6.  Scan output at slot n is
                # then exactly c_in of nibble n.
                NJ = ntc * 8
                zt_tn = zt[:, 0 : NJ].rearrange("p (t n) -> p t n", n=8)
                d0 = apool.tile([P, 8 * MAX_NTC], FP, tag="d0")
                d1 = apool.tile([P, 8 * MAX_NTC], FP, tag="d1")
                c_pre = apool.tile([P, 8 * MAX_NTC], FP, tag="cpre")
                d0_v = d0[:, 0:NJ].rearrange("p (t s) -> p t s", s=8)
                d1_v = d1[:, 0:NJ].rearrange("p (t s) -> p t s", s=8)
                nc.vector.memset(d0[:, 0:NJ], 0.5)
                nc.vector.memset(d1[:, 0:NJ], 0.5)
                nc.vector.tensor_scalar(
                    out=d0_v[:, :, 1:8], in0=zt_tn[:, :, 0:7], scalar1=14.5,
                    scalar2=None, op0=AL.is_ge,
                )
                nc.vector.tensor_scalar(
                    out=d1_v[:, :, 1:8], in0=zt_tn[:, :, 0:7], scalar1=15.5,
                    scalar2=None, op0=AL.is_ge,
                )
                nc.vector.tensor_tensor_scan(
                    out=c_pre[:, 0:NJ], data0=d0[:, 0:NJ], data1=d1[:, 0:NJ],
                    initial=0.5, op0=AL.min, op1=AL.max,
                )

                # ---- vectorized U/P/weights ----
                # flat ops in zt's (t, n)=(t, i, par) layout; u/w0/w1 are
                # written through strided views into (n, t) n-major storage
                # so the dist build gets packed last dims (2x DVE mode).
                p_all = apool.tile([P, 8 * MAX_NTC], FP, tag="pall")
                y_all = apool.tile([P, 8 * MAX_NTC], FP, tag="yall")
                wrap = apool.tile([P, 8 * MAX_NTC], FP, tag="wrap")
                u_all = apool.tile([P, 8 * MAX_NTC], BF, tag="uall")
                w0_all = apool.tile([P, 8 * MAX_NTC], BF, tag="w0")
                w1_all = apool.tile([P, 8 * MAX_NTC], BF, tag="w1")

                def tip(t_ap):  # (t,i,par)-flat tensor -> view [p, t, i, par]
                    return t_ap[:, 0:NJ].rearrange(
                        "p (t i par) -> p t i par", t=ntc, i=4, par=2
                    )

                def pti(t_ap):  # (n,t)=(i,par,t) storage -> [p, t, i, par]
                    return t_ap[:, 0:NJ].rearrange(
                        "p (i par t) -> p t i par", i=4, par=2, t=ntc
                    )

                # y = x + (c_pre >= 0.75), fused into one STT
                nc.vector.scalar_tensor_tensor(
                    out=y_all[:, 0:NJ], in0=c_pre[:, 0:NJ], scalar=0.75,
                    in1=zt[:, 0:NJ], op0=AL.is_ge, op1=AL.add,
                )
                nc.vector.tensor_scalar(
                    out=wrap[:, 0:NJ], in0=y_all[:, 0:NJ], scalar1=15.5,
                    scalar2=None, op0=AL.is_ge,
                )
                nc.vector.scalar_tensor_tensor(
                    out=pti(u_all), in0=tip(wrap), scalar=-16.0,
                    in1=tip(y_all), op0=AL.mult, op1=AL.add,
                )
                nc.vector.tensor_scalar(
                    out=p_all[:, 0:NJ], in0=c_pre[:, 0:NJ], scalar1=0.5,
                    scalar2=None, op0=AL.is_equal,
                )
                nc.vector.tensor_scalar(
                    out=pti(w1_all), in0=tip(p_all), scalar1=0.5,
                    scalar2=None, op0=AL.mult,
                )
                nc.vector.tensor_scalar(
                    out=pti(w0_all), in0=tip(p_all), scalar1=-0.5,
                    scalar2=1.0, op0=AL.mult, op1=AL.add,
                )

                # ---- dist + outers per 4-tile sub (= output quad) ----
                # dist layout (k, n, t): packed t -> 2x DVE mode, and the
                # outer reads keep the measured-fast stride pattern
                # [i:16B, h:64B, l:0].
                u_nv = u_all[:, 0:NJ].rearrange("p (n t) -> p n t", n=8)
                w0_nv = w0_all[:, 0:NJ].rearrange("p (n t) -> p n t", n=8)
                w1_nv = w1_all[:, 0:NJ].rearrange("p (n t) -> p n t", n=8)
                iota_f = iota[:, :].rearrange("p (k n t) -> p k n t", k=17, n=8)
                subs = [(o, min(4, ntc - o)) for o in range(0, ntc, 4)]
                for ts0, st in subs:
                    iota_v = iota_f[:, :, :, 0:st]
                    eqx = dpool.tile([P, 17 * 32], BF, tag="eqx")
                    dsub = dpool.tile([P, 16 * 32], BF, tag="dsub")
                    dtmp = dpool.tile([P, 16 * 32], BF, tag="dtmp")
                    eqx_v = eqx[:, 0 : 17 * 8 * st].rearrange(
                        "p (k n t) -> p k n t", k=17, n=8
                    )
                    dsub_v = dsub[:, 0 : 16 * 8 * st].rearrange(
                        "p (k n t) -> p k n t", k=16, n=8
                    )
                    dtmp_v = dtmp[:, 0 : 16 * 8 * st].rearrange(
                        "p (k n t) -> p k n t", k=16, n=8
                    )
                    u_b = u_nv[:, None, :, ts0 : ts0 + st].broadcast_to([P, 17, 8, st])
                    w0_b = w0_nv[:, None, :, ts0 : ts0 + st].broadcast_to([P, 16, 8, st])
                    w1_b = w1_nv[:, None, :, ts0 : ts0 + st].broadcast_to([P, 16, 8, st])
                    nc.vector.tensor_tensor(eqx_v, u_b, iota_v, op=AL.is_equal)
                    nc.vector.tensor_mul(dsub_v, eqx_v[:, 1:17], w0_b)
                    nc.vector.tensor_mul(dtmp_v, eqx_v[:, 0:16], w1_b)
                    nc.vector.tensor_add(
                        dsub[:, 0 : 16 * 8 * st],
                        dsub[:, 0 : 16 * 8 * st],
                        dtmp[:, 0 : 16 * 8 * st],
                    )

                    # dv: [p, k(nibble value), i(byte), par(lo/hi), t]
                    dv = dsub[:, 0 : 16 * 8 * st].rearrange(
                        "p (k i par t) -> p k i par t", k=16, i=4, par=2, t=st
                    )
                    # gather the l-dists into contiguous (t, i, l) once, so
                    # every l_rep below reads a PACKED last dim and hits the
                    # fast DVE copy mode (strided-read copies measure ~1.0
                    # ns/elem, packed ones ~0.4)
                    fl2 = dpool.tile([P, 4 * 64], BF, tag="fl2")
                    nc.vector.tensor_copy(
                        fl2[:, 0 : st * 64].rearrange(
                            "p (t i l) -> p t i l", i=4, l=16
                        ),
                        dv[:, :, :, 0, :].rearrange("p k i t -> p t i k"),
                    )
                    # Outers: a broadcast-AP TT measures ~1.85 ns/elem on DVE
                    # regardless of dtype, while copies/TS with arbitrary
                    # strides run in the 2x_2p mode (~0.55) and packed bf16
                    # TTs in 2x_1p.  So materialize the replicated factors --
                    # h_rep on the mostly-idle Act engine, l_rep on DVE --
                    # and do one all-packed bf16 multiply per tile.
                    o4 = opool.tile([P, 4 * FOUT], BF, tag="o4")
                    hrep = opool.tile([P, 4 * FOUT], BF, tag="hrep")
                    lrep = opool.tile([P, 4 * FOUT], BF, tag="lrep")
                    for tl in range(st):
                        h_t = (
                            dv[:, :, :, 1, tl]
                            .rearrange("p k i -> p i k")[:, :, :, None]
                            .broadcast_to([P, 4, 16, 16])
                        )
                        l_t = (
                            fl2[:, tl * 64 : (tl + 1) * 64]
                            .rearrange("p (i l) -> p i l", i=4)[:, :, None, :]
                            .broadcast_to([P, 4, 16, 16])
                        )
                        hrep_v = hrep[:, tl * FOUT : (tl + 1) * FOUT].rearrange(
                            "p (i h l) -> p i h l", h=16, l=16
                        )
                        lrep_v = lrep[:, tl * FOUT : (tl + 1) * FOUT].rearrange(
                            "p (i h l) -> p i h l", h=16, l=16
                        )
                        nc.scalar.copy(hrep_v, h_t)
                        nc.vector.tensor_copy(lrep_v, l_t)
                    for m0 in range(0, st, 2):
                        mw = min(2, st - m0)
                        nc.vector.tensor_mul(
                            o4[:, m0 * FOUT : (m0 + mw) * FOUT],
                            hrep[:, m0 * FOUT : (m0 + mw) * FOUT],
                            lrep[:, m0 * FOUT : (m0 + mw) * FOUT],
                        )
                    t_first = t_glob + ts0
                    if st == 4:
                        o_view = out4_v[t_first // 4]
                    elif st == 2:
                        o_view = out2_v[t_first // 2]
                    else:
                        o_view = out1_v[t_first]
                    if last_ch:
                        nc.scalar.dma_start(o_view, o4[:, 0 : st * FOUT])
                    else:
                        pending_outs.append((o_view, o4, st))

                t_glob += ntc

            for o_view, o4p, sz in pending_outs:
                nc.scalar.dma_start(o_view, o4p[:, 0 : sz * FOUT])

    nc.finalize()
    return nc


_NC_CACHE = {}
LAST_RESULT = None


def kernel(**inputs) -> np.ndarray:
    global LAST_RESULT
    a = np.asarray(inputs["a"], dtype=np.float32).reshape(B_FULL, 1024)
    b = np.asarray(inputs["b"], dtype=np.float32).reshape(B_FULL, 1024)
    ab = np.concatenate([a, b], axis=1).astype(F8_NP)  # [B, 2048] fp8, exact
    wtab, iota_rep, ident8 = _const_tables()

    if "nc" not in _NC_CACHE:
        _NC_CACHE["nc"] = build_nc()
    nc = _NC_CACHE["nc"]

    in_maps = []
    for c in range(N_CORES):
        abT = np.ascontiguousarray(ab[c * ROWS : (c + 1) * ROWS].T)  # [2048, 4096]
        in_maps.append({
            "abT": abT,
            "wtab": wtab,
            "iota_rep": iota_rep,
            "ident8": ident8,
        })
    res = run_bass_kernel_spmd(nc, in_maps, core_ids=list(range(N_CORES)))
    LAST_RESULT = res
    out = np.concatenate([r["out"] for r in res.results], axis=0)  # bf16
    return out.astype(np.float32).reshape(B_FULL, 4, 256)
